# revision 8
# baseline (speedup 1.0000x reference)
"""Trainium2 Bass kernel for causal multi-head attention block (B=8, S=1024, D=1024, H=16).

Sharding: pure batch data-parallelism - one batch element per NeuronCore (B=8, 8 cores).
Each core runs the full transformer block on its [S, D] slice; no collectives.

Per-core algorithm (layouts chosen so no on-device transposes are needed):
  - Host passes x^T and all W^T pre-strided into the SBUF partition layout
    [p, db, cols] so every big DMA is 128 large contiguous descriptors.
  - Q^T, K^T computed as [o, s] via matmul(lhsT=W^T block, rhs=x^T); o = h*64+dk on
    partitions -> per-head [dk, S] slices feed the scores matmul directly.
  - V computed natural [s, o] via matmul(lhsT=x^T block, rhs=Wv^T); stored per head
    PAIR as [even64 | one_e | one_o | odd64] (130 cols/pair) so that
      * even-head ctx matmul (128-wide window at pair base) puts dims on PSUM
        partitions 0..63 and s0 (from the ones column) on partition 64;
      * odd-head ctx matmul (window at base+2) puts s0 on partition 63 and dims on
        partitions 64..127 - both heads of a pair land on their final partitions,
        no partition-shifting copies needed.
  - scoresT[k, q] = matmul(lhsT=K^T head slice, rhs=Q^T head slice) (contraction
    dk=64; even/odd heads at base partitions 0/64 -> concurrent PE row groups).
    k-blocks processed in pairs sharing one 2-bank PSUM tile.
  - Softmax without max-subtraction: u = exp(0.125*scores) on ACT. Padded keys are
    handled by zeroed V' rows; the causal mask is a 0/1 multiply on the diagonal
    blocks (split across DVE and gpsimd).
  - The attention tiles (head-pair x q-half) are software-pipelined: per step we
    emit ctx(i-1) -> scores(i)+exp(i) -> normalize(i-1), so the exp/mask latency of
    a tile hides under the next tile's PE stream and the PE never stalls.
  - Per-tile softmax normalization: s0 rows are vector-copied (same partition) from
    ctx PSUM into a staging tile, denom/bad/recip computed on 2 partitions, the odd
    row moved 63->96 by a tiny SBUF DMA, and a single K=33 selector matmul
    broadcasts recip/bad to all 128 partitions. Rows with a fully-masked causal
    window (s0 == 0) match the reference's softmax(-1e9*ones) = uniform over ALL
    1024 keys: ctx = (ctx_u + bad*sumV)/(s0 + 1024*bad), bad = (s0 <= 1e-30).
    The normalize is fused into the PSUM->SBUF move (scalar_tensor_tensor + mult).
  - Out projection: out[s, o] = matmul(lhsT=ctx^T block, rhs=Wo^T); residual x+bo
    added on DVE, LayerNorm stats via bn_stats/bn_aggr, final normalize on ACT
    (scale=rstd, bias=-mu*rstd). gamma/beta applied on host (exact no-op for the
    reference's ones/zeros).
"""

import sys

import numpy as np

if "/opt/trn_rl_repo" not in sys.path:
    sys.path.insert(0, "/opt/trn_rl_repo")

S = 1024
D = 1024
H = 16
DK = 64
P = 128
DB = D // P  # 8 d-blocks
SB = S // P  # 8 s-blocks
NEG = -1.0e9
SCALE = 0.125  # 1/sqrt(64)
EPS = 1e-5
N_CORES = 8
PW = 2 * DK + 2  # 130: per-pair width in V'

_built = None


def _kbs(qc):
    """k-block pairs needed for q-chunk qc (q in [qc*512, qc*512+512))."""
    return [(0, 1), (2, 3)] if qc == 0 else [(0, 1), (2, 3), (4, 5), (6, 7)]


def _vs(kb, qc):
    """first causal q column within the 512-wide chunk for k-block kb."""
    return max(0, kb * P - qc * 512)


def _build():
    import concourse.mybir as mybir
    import concourse.tile as tile
    from concourse import bacc

    f32 = mybir.dt.float32
    f32r = mybir.dt.float32r
    Alu = mybir.AluOpType
    Act = mybir.ActivationFunctionType

    nc = bacc.Bacc()
    fmm = f32r

    # ---- DRAM I/O (pre-strided [p, db, cols] contiguous layouts from host) ----
    xt_lo_d = nc.dram_tensor("xt_lo", [P, 4, S], fmm, kind="ExternalInput")
    xt_hi_d = nc.dram_tensor("xt_hi", [P, 4, S], fmm, kind="ExternalInput")
    xr_d = nc.dram_tensor("xr", [S, D], f32, kind="ExternalInput")  # x + bo
    w_d = {}
    for wname in ("wq", "wk", "wv", "wo"):
        for oc in range(2):
            n = f"{wname}{oc}"
            w_d[n] = nc.dram_tensor(n, [P, DB, 512], fmm, kind="ExternalInput")
    bqs_d = nc.dram_tensor("bqs", [P, DB], f32, kind="ExternalInput")
    bks_d = nc.dram_tensor("bks", [P, DB], f32, kind="ExternalInput")
    padm_d = nc.dram_tensor("padm", [P, SB], f32, kind="ExternalInput")  # 1 valid / 0 pad
    sumv_d = nc.dram_tensor("sumv", [P, DB], f32, kind="ExternalInput")
    sel_d = nc.dram_tensor("sel", [P, P], fmm, kind="ExternalInput")
    causal_d = nc.dram_tensor("causal", [P, P], f32, kind="ExternalInput")  # 0/1
    out_d = nc.dram_tensor("out", [S, D], f32, kind="ExternalOutput")

    with tile.TileContext(nc) as tc:
        with (
            tc.tile_pool(name="singles", bufs=1) as singles,
            tc.tile_pool(name="qt", bufs=1) as qt_pool,
            tc.tile_pool(name="kt", bufs=1) as kt_pool,
            tc.tile_pool(name="vp", bufs=1) as vp_pool,
            tc.tile_pool(name="big16", bufs=4) as big16,  # 16KB: x^T halves, then u
            tc.tile_pool(name="c2m", bufs=2) as c2m,  # 16KB weight chunks
            tc.tile_pool(name="xres", bufs=2) as xres_pool,
            tc.tile_pool(name="small", bufs=2) as small,
            tc.tile_pool(name="psS", bufs=3, space="PSUM") as psS,  # 2-bank tiles
            tc.tile_pool(name="psC", bufs=2, space="PSUM") as psC,  # 1-bank ctx tiles
        ):
            # persistent big tensors; first weight stripes load interleaved with x^T
            # (weights on Sync queue, x^T on Activation queue) so the first
            # projection matmuls start as early as possible.
            xT_lo = big16.tile([P, 4, S], fmm, tag="big16")
            xT_hi = big16.tile([P, 4, S], fmm, tag="big16")
            wch0 = c2m.tile([P, DB, 512], fmm, tag="c2m", name="wch0")
            for i in range(4):
                nc.sync.dma_start(wch0[:, 2 * i : 2 * i + 2, :], w_d["wq0"][:, 2 * i : 2 * i + 2, :])
                nc.scalar.dma_start(xT_lo[:, i, :], xt_lo_d[:, i, :])
            for i in range(4):
                nc.scalar.dma_start(xT_hi[:, i, :], xt_hi_d[:, i, :])

            def xT_sb(db, sl):
                t = xT_lo if db < 4 else xT_hi
                return t[:, db % 4, sl]

            QT = qt_pool.tile([P, DB, S], fmm, tag="qt")  # later overlaid with ctx^T
            KT = kt_pool.tile([P, DB, S], fmm, tag="kt")
            Vp = vp_pool.tile([P, SB, DB * PW], fmm, tag="vp")

            # ---- constants / singles ----
            bq_sb = singles.tile([P, DB], f32)
            nc.sync.dma_start(bq_sb[:], bqs_d[:, :])
            bk_sb = singles.tile([P, DB], f32)
            nc.sync.dma_start(bk_sb[:], bks_d[:, :])
            eps_sb = singles.tile([P, 1], f32)
            nc.vector.memset(eps_sb[:], EPS)
            sel_sb = singles.tile([P, P], fmm)
            nc.sync.dma_start(sel_sb[:], sel_d[:, :])
            # s0 staging/normalizer tile. Engine APs must start at partition
            # 0/32/64/96, so: even s0 -> row 64 directly; odd s0 (PSUM row 63)
            # arrives via an aligned [32:64) copy, then a 1-partition SBUF DMA
            # moves row 63 -> 96. The denom/bad/recip chain runs on the aligned
            # span [64:97); rows 65..95 are initialized to 1.0 once so the chain
            # keeps them at finite 1.0 (they meet zero selector weights in the
            # broadcast matmul, and 0*inf would poison it).
            ones_f32 = singles.tile([P, 1], f32)
            nc.vector.memset(ones_f32[:], 1.0)
            stg = singles.tile([P, 2, 512], fmm)
            nc.vector.tensor_scalar(
                stg[:, 0, :], ones_f32.to_broadcast([P, 512]), 1.0, None, op0=Alu.mult
            )

            # ============ Phase 1: projections ============
            # Q^T / K^T: psum[o_block 128, s 1024] = sum_db WT[db, ob].T @ xT[db, :]
            for wname, dst, bias_sb in (("wq", QT, bq_sb), ("wk", KT, bk_sb)):
                for oc in range(2):
                    if wname == "wq" and oc == 0:
                        wch = wch0
                    else:
                        wch = c2m.tile([P, DB, 512], fmm, tag="c2m")
                        nc.sync.dma_start(wch[:, 0:4, :], w_d[f"{wname}{oc}"][:, 0:4, :])
                        nc.sync.dma_start(wch[:, 4:8, :], w_d[f"{wname}{oc}"][:, 4:8, :])
                    for obl in range(4):
                        ob = oc * 4 + obl
                        ps = psS.tile([P, 2, 512], f32, tag="mm")
                        for sc in range(2):
                            for db in range(DB):
                                nc.tensor.matmul(
                                    ps[:, sc, :],
                                    lhsT=wch[:, db, obl * P : (obl + 1) * P],
                                    rhs=xT_sb(db, slice(sc * 512, (sc + 1) * 512)),
                                    start=(db == 0),
                                    stop=(db == DB - 1),
                                )
                        # copy + per-partition bias (o on partitions), both chunks
                        nc.scalar.activation(
                            dst[:, ob, :],
                            ps[:].rearrange("p a b -> p (a b)"),
                            Act.Identity,
                            bias=bias_sb[:, ob : ob + 1],
                        )

            # late singles (not needed until V-proj / attention / epilogue)
            padm_sb = singles.tile([P, SB], f32)
            nc.sync.dma_start(padm_sb[:], padm_d[:, :])
            causal_sb = singles.tile([P, P], f32)
            nc.sync.dma_start(causal_sb[:], causal_d[:, :])
            sumv_all = singles.tile([P, DB], f32)
            nc.sync.dma_start(sumv_all[:], sumv_d[:, :])

            # V natural: psum[s_block 128, o 512] = sum_db xT[db, sb].T @ WvT[db, oc]
            for oc in range(2):
                wch = c2m.tile([P, DB, 512], fmm, tag="c2m")
                nc.sync.dma_start(wch[:, 0:4, :], w_d[f"wv{oc}"][:, 0:4, :])
                nc.sync.dma_start(wch[:, 4:8, :], w_d[f"wv{oc}"][:, 4:8, :])
                for sbi in range(0, SB, 2):
                    ps = psS.tile([P, 2, 512], f32, tag="mm")
                    for si in range(2):
                        sb = sbi + si
                        for db in range(DB):
                            nc.tensor.matmul(
                                ps[:, si, :],
                                lhsT=xT_sb(db, slice(sb * P, (sb + 1) * P)),
                                rhs=wch[:, db, :],
                                start=(db == 0),
                                stop=(db == DB - 1),
                            )
                    for si in range(2):
                        sb = sbi + si
                        # scatter into per-pair slots + key-padding zeroing
                        vview = Vp[:, sb, :].rearrange("p (pr c) -> p pr c", c=PW)
                        sview = ps[:, si, :].rearrange("p (pr e c) -> p pr e c", e=2, c=DK)
                        prs = slice(oc * 4, (oc + 1) * 4)
                        nc.vector.tensor_scalar(
                            vview[:, prs, 0:DK],
                            sview[:, :, 0, :],
                            padm_sb[:, sb : sb + 1],
                            None,
                            op0=Alu.mult,
                        )
                        nc.vector.tensor_scalar(
                            vview[:, prs, DK + 2 : PW],
                            sview[:, :, 1, :],
                            padm_sb[:, sb : sb + 1],
                            None,
                            op0=Alu.mult,
                        )
            # "ones" columns of V' = padmask (zero for padded keys)
            vv = Vp[:, :, :].rearrange("p sb (pr c) -> p sb pr c", c=PW)
            nc.vector.tensor_copy(
                vv[:, :, :, DK : DK + 2],
                padm_sb.unsqueeze(2).unsqueeze(3).to_broadcast([P, SB, DB, 2]),
            )

            # ============ Phase 2: attention (software-pipelined tiles) ============
            tiles = [(0, hb) for hb in range(8)] + [(1, hb) for hb in range(8)]

            def emit_scores(qc, hb):
                uts = {}
                for par in range(2):
                    uts[par] = big16.tile([P, DB, 512], fmm, tag="big16", name=f"ut{par}")
                for kb0, kb1 in _kbs(qc):
                    vs = _vs(kb0, qc)  # pair shares the lower block's start col
                    for par in range(2):
                        hp = 64 * par
                        ps = psS.tile([P, 2, 512], f32, tag="mm")
                        for i, kb in enumerate((kb0, kb1)):
                            nc.tensor.matmul(
                                ps[:, i, vs:512],
                                lhsT=KT[hp : hp + DK, hb, kb * P : (kb + 1) * P],
                                rhs=QT[hp : hp + DK, hb, qc * 512 + vs : qc * 512 + 512],
                                start=True,
                                stop=True,
                            )
                        # u = exp(0.125*scores); padding handled by zeroed V rows
                        nc.scalar.activation(
                            uts[par][:, kb0 : kb0 + 2, vs:512],
                            ps[:, :, vs:512],
                            Act.Exp,
                            scale=SCALE,
                        )
                        eng = nc.vector if par == 0 else nc.gpsimd
                        for kb in (kb0, kb1):
                            if kb * P >= qc * 512:  # diagonal: 0/1 causal mask on u
                                dvs = _vs(kb, qc)
                                eng.tensor_mul(
                                    uts[par][:, kb, dvs : dvs + P],
                                    uts[par][:, kb, dvs : dvs + P],
                                    causal_sb[:],
                                )
                return uts

            def emit_ctx(qc, hb, uts):
                cps = {}
                klist = [kb for pr in _kbs(qc) for kb in pr]
                for par in range(2):
                    cps[par] = psC.tile([P, 512], f32, tag="ctx", name=f"cps{par}")
                    base = hb * PW + (0 if par == 0 else 2)
                    for i, kb in enumerate(klist):
                        vs = _vs(kb, qc)
                        nc.tensor.matmul(
                            cps[par][:, vs:512],
                            lhsT=Vp[:, kb, base : base + P],
                            rhs=uts[par][:, kb, vs:512],
                            start=(i == 0),
                            stop=(i == len(klist) - 1),
                        )
                # s0 rows: even head on PSUM partition 64, odd head on partition 63
                nc.vector.tensor_copy(stg[64:65, 0, :], cps[0][64:65, 0:512])
                nc.vector.tensor_copy(stg[32:64, 0, :], cps[1][32:64, 0:512])
                # odd row 63 -> 96 (DMA has no partition-alignment restriction)
                nc.sync.dma_start(stg[96:97, 0, :], stg[63:64, 0, :])
                # denom/bad/recip on aligned span [64:97): bad1024 =
                # (s0<=1e-30)*1024, s0 += bad1024, recip = 1/s0
                nc.vector.tensor_scalar(
                    stg[64:97, 1, :], stg[64:97, 0, :], 1e-30, 1024.0,
                    op0=Alu.is_le, op1=Alu.mult,
                )
                nc.vector.tensor_tensor(
                    stg[64:97, 0, :], stg[64:97, 0, :], stg[64:97, 1, :], Alu.add
                )
                with nc.allow_low_precision(
                    reason="recip stored f32r; f32r rounding far below output tolerance"
                ):
                    nc.vector.reciprocal(stg[64:97, 0, :], stg[64:97, 0, :])
                return cps

            def emit_norm(qc, hb, cps):
                # broadcast recip (col 0) + bad1024 (col 1) from partitions 64/96
                # to all 128: sel[64,p]=p<64, sel[96,p]=p>=64, zeros elsewhere.
                bc2 = psS.tile([P, 2, 512], f32, tag="mm", name="bc2")
                nc.tensor.matmul(
                    bc2[:, 0, :], lhsT=sel_sb[64:97, :], rhs=stg[64:97, 1, :],
                    start=True, stop=True,
                )
                nc.tensor.matmul(
                    bc2[:, 1, :], lhsT=sel_sb[64:97, :], rhs=stg[64:97, 0, :],
                    start=True, stop=True,
                )
                qch = slice(qc * 512, (qc + 1) * 512)
                # raw ctx -> QT (even head partitions 0..63, odd 64..127; both
                # pars land on their final partitions, no partition shift), then
                # ctx = (ctx_u + bad1024 * sumV/1024) * recip in place. An op may
                # read at most one PSUM operand, so bc2 stays the only PSUM src.
                nc.vector.tensor_copy(QT[0:64, hb, qch], cps[0][0:64, :])
                nc.vector.tensor_copy(QT[64:128, hb, qch], cps[1][64:128, :])
                nc.vector.scalar_tensor_tensor(
                    QT[:, hb, qch], bc2[:, 0, :], sumv_all[:, hb : hb + 1],
                    QT[:, hb, qch], op0=Alu.mult, op1=Alu.add,
                )
                nc.vector.tensor_tensor(
                    QT[:, hb, qch], QT[:, hb, qch], bc2[:, 1, :], Alu.mult
                )

            prev = None
            woch = []
            for i, (qc, hb) in enumerate(tiles):
                if prev is not None:
                    pqc, phb, puts = prev
                    pcps = emit_ctx(pqc, phb, puts)
                uts = emit_scores(qc, hb)
                if i == 1:
                    # wo prefetch: c2m slots are free after V-proj; DMA overlaps
                    # the attention phase
                    for oc in range(2):
                        wch = c2m.tile([P, DB, 512], fmm, tag="c2m")
                        nc.sync.dma_start(wch[:, 0:4, :], w_d[f"wo{oc}"][:, 0:4, :])
                        nc.sync.dma_start(wch[:, 4:8, :], w_d[f"wo{oc}"][:, 4:8, :])
                        woch.append(wch)
                if prev is not None:
                    emit_norm(pqc, phb, pcps)
                prev = (qc, hb, uts)

            # drain the pipeline; out-proj sb=0 matmuls fill the PE while the last
            # tile's s0 chain completes
            pqc, phb, puts = prev
            pcps = emit_ctx(pqc, phb, puts)

            # ============ Phase 3: out-projection + residual + LayerNorm ============
            def emit_outproj(sb):
                xres = xres_pool.tile([P, D], f32, tag="xres")
                nc.sync.dma_start(xres[:], xr_d[sb * P : (sb + 1) * P, :])
                ps = psS.tile([P, 2, 512], f32, tag="mm")
                for oc in range(2):
                    for db in range(DB):
                        nc.tensor.matmul(
                            ps[:, oc, :],
                            lhsT=QT[:, db, sb * P : (sb + 1) * P],
                            rhs=woch[oc][:, db, :],
                            start=(db == 0),
                            stop=(db == DB - 1),
                        )
                nc.vector.tensor_add(
                    xres[:, :], ps[:].rearrange("p a b -> p (a b)"), xres[:, :]
                )
                # LayerNorm over free dim (1024) via bn_stats (2 subgroups of 512)
                stats = small.tile([P, 2, 6], f32, tag="stats")
                nc.vector.bn_stats(stats[:, 0, :], xres[:, 0:512])
                nc.vector.bn_stats(stats[:, 1, :], xres[:, 512:1024])
                mv = small.tile([P, 2], f32, tag="mv")
                nc.vector.bn_aggr(mv[:], stats[:])
                rstd = small.tile([P, 2], f32, tag="rstd")
                nc.scalar.activation(
                    rstd[:, 0:1], mv[:, 1:2], Act.Sqrt, bias=eps_sb[:], scale=1.0
                )
                nc.vector.reciprocal(rstd[:, 0:1], rstd[:, 0:1])
                # -mu * rstd
                nc.vector.tensor_scalar(
                    rstd[:, 1:2], mv[:, 0:1], -1.0, rstd[:, 0:1],
                    op0=Alu.mult, op1=Alu.mult,
                )
                # final normalize on ACT: out = res*rstd + (-mu*rstd)
                nc.scalar.activation(
                    xres[:, :], xres[:, :], Act.Identity,
                    bias=rstd[:, 1:2], scale=rstd[:, 0:1],
                )
                nc.sync.dma_start(out_d[sb * P : (sb + 1) * P, :], xres[:])

            emit_outproj(0)
            emit_norm(pqc, phb, pcps)
            for sb in range(1, SB):
                emit_outproj(sb)

    nc.compile()
    return nc


def _stripe_w(WT):
    """[D, D] (d_in, d_out) -> two contiguous [P, DB, 512] o-half chunks."""
    a = np.ascontiguousarray(WT.reshape(DB, P, D).transpose(1, 0, 2))  # [p, db, o]
    return (
        np.ascontiguousarray(a[:, :, 0:512]),
        np.ascontiguousarray(a[:, :, 512:1024]),
    )


def kernel(
    history_items,
    sequence_mask,
    Wq,
    bq,
    Wk,
    bk,
    Wv,
    bv,
    Wo,
    bo,
    ln_gamma,
    ln_beta,
):
    from concourse.bass_utils import run_bass_kernel_spmd

    global _built
    if _built is None:
        _built = _build()
    nc = _built

    x = np.asarray(history_items, dtype=np.float32)
    mask = np.asarray(sequence_mask)
    f = lambda a: np.ascontiguousarray(np.asarray(a, dtype=np.float32))

    common = {}
    for wname, W in (("wq", Wq), ("wk", Wk), ("wv", Wv), ("wo", Wo)):
        c0, c1 = _stripe_w(f(np.asarray(W).T))
        common[f"{wname}0"] = c0
        common[f"{wname}1"] = c1
    common["bqs"] = f(np.asarray(bq).reshape(DB, P).T)
    common["bks"] = f(np.asarray(bk).reshape(DB, P).T)
    sel = np.zeros((P, P), dtype=np.float32)
    sel[64, 0:64] = 1.0
    sel[96, 64:128] = 1.0
    common["sel"] = sel
    common["causal"] = f(
        np.where(np.arange(P)[None, :] >= np.arange(P)[:, None], 1.0, 0.0)
    )
    # attn-output bias bv contributes bv @ Wo.T (constant over s) -> fold into residual
    bo_row = (
        np.asarray(bo, dtype=np.float64)
        + np.asarray(bv, dtype=np.float64) @ np.asarray(Wo, dtype=np.float64).T
    ).astype(np.float32)

    in_maps = []
    for b in range(N_CORES):
        xT = f(x[b].T).reshape(DB, P, S).transpose(1, 0, 2)  # [p, db, s]
        pm = (mask[b] != 0).astype(np.float32)
        sx = x[b].astype(np.float64).sum(axis=0)
        sumv = ((sx @ np.asarray(Wv, dtype=np.float64).T) / 1024.0).astype(np.float32)
        in_maps.append(
            {
                **common,
                "xt_lo": np.ascontiguousarray(xT[:, 0:4, :]),
                "xt_hi": np.ascontiguousarray(xT[:, 4:8, :]),
                "xr": f(x[b] + bo_row[None, :]),
                "padm": f(pm.reshape(SB, P).T),
                "sumv": f(sumv.reshape(DB, P).T),
            }
        )

    r = run_bass_kernel_spmd(nc, in_maps, core_ids=list(range(N_CORES)))
    out = np.stack([res["out"] for res in r.results]).astype(np.float32)

    g = np.asarray(ln_gamma, dtype=np.float32)
    be = np.asarray(ln_beta, dtype=np.float32)
    out = out * g[None, None, :] + be[None, None, :]
    return out.astype(np.float32)
